# revision 5
# baseline (speedup 1.0000x reference)
"""GAE (segment-softmax pooling) + FiLM kernel for 8 trn2 NeuronCores.

Sharding: graphs are padded to a uniform length L and laid out one graph per
SBUF partition row; 128 graphs per core x 8 cores = 1024 graphs. All segment
reductions become free-dim reductions, fully device-local (no collectives).
"""

import os
import numpy as np

_NCORES = 8
_GPC = 128  # graphs per core
_Z = 32
_H = 64

_prog_cache = {}
_last_results = None  # test.py introspects this for profiling info


def _build_program(L, Lc, wg_vals, bg_val):
    """One SPMD program, run on all 8 cores with different input slabs.

    Per-core tensors:
      zin  [128, L, 32] f32  - padded node features, one graph per partition
      w1b  [33, 64] f32      - [W1; b1]
      w2b  [65, 64] f32      - [W2; b2]
      ident [128, 128] f32   - identity for PE transposes
      zmod [128, L, 32] f32  - output (padded layout)
      gout [128, 32] f32     - per-graph pooled features
    """
    import concourse.bass as bass
    import concourse.mybir as mybir
    from concourse import bacc, tile

    f32 = mybir.dt.float32
    AF = mybir.ActivationFunctionType
    OP = mybir.AluOpType
    AX = mybir.AxisListType
    n_chunks = L // Lc
    assert n_chunks * Lc == L

    nc = bacc.Bacc("TRN2", target_bir_lowering=False, debug=False)

    zin = nc.dram_tensor("zin", [_GPC, L, _Z], f32, kind="ExternalInput")
    w1b = nc.dram_tensor("w1b", [_Z + 1, _H], f32, kind="ExternalInput")
    w2b = nc.dram_tensor("w2b", [_H + 1, 2 * _Z], f32, kind="ExternalInput")
    ident = nc.dram_tensor("ident", [128, 128], f32, kind="ExternalInput")
    zmod = nc.dram_tensor("zmod", [_GPC, L, _Z], f32, kind="ExternalOutput")
    gout = nc.dram_tensor("gout", [_GPC, _Z], f32, kind="ExternalOutput")

    with tile.TileContext(nc) as tc:
        with (
            tc.tile_pool(name="zbuf", bufs=2) as zpool,
            tc.tile_pool(name="ebuf", bufs=2) as epool,
            tc.tile_pool(name="small", bufs=1) as small,
            tc.tile_pool(name="psum", bufs=1, space=bass.MemorySpace.PSUM) as psum,
        ):
            denom_parts = small.tile([_GPC, n_chunks], f32)
            gsum_parts = small.tile([_GPC, _Z * n_chunks], f32)
            w1b_sb = small.tile([_Z + 1, _H], f32)
            w2b_sb = small.tile([_H + 1, 2 * _Z], f32)
            ident_sb = small.tile([128, 128], f32)
            nc.sync.dma_start(w1b_sb[:], w1b[:])
            nc.sync.dma_start(w2b_sb[:], w2b[:])
            nc.sync.dma_start(ident_sb[:], ident[:])
            bg_col = small.tile([_GPC, 1], f32)
            nc.vector.memset(bg_col[:], float(bg_val))

            # ---- pass 1: gate -> e -> per-graph partial sums ----
            for c in range(n_chunks):
                zt = zpool.tile([_GPC, Lc, _Z], f32, tag="z")
                nc.sync.dma_start(zt[:], zin[:, c * Lc : (c + 1) * Lc, :])
                # gate accumulation: acc = sum_f z[:, :, f] * wg[f]
                acc = epool.tile([_GPC, Lc], f32, tag="acc")
                nc.vector.tensor_scalar_mul(acc[:], zt[:, :, 0], float(wg_vals[0]))
                for f in range(1, _Z):
                    nc.vector.scalar_tensor_tensor(
                        acc[:], zt[:, :, f], float(wg_vals[f]), acc[:],
                        OP.mult, OP.add,
                    )
                # e = exp(gate + bg); denom partial via accum_out
                e = epool.tile([_GPC, Lc], f32, tag="e")
                nc.scalar.activation(
                    e[:], acc[:], AF.Exp, bias=bg_col[:, 0:1],
                    accum_out=denom_parts[:, c : c + 1],
                )
                # gsum partials: sum_t e * z_f per graph, one fused op per f
                # (tensor_tensor_reduce crashes the exec unit on this runtime;
                #  scalar_tensor_tensor with accum_out computes the same thing)
                scratch = epool.tile([_GPC, Lc], f32, tag="scr")
                for f in range(_Z):
                    nc.vector.scalar_tensor_tensor(
                        scratch[:], zt[:, :, f], 1.0, e[:],
                        OP.mult, OP.mult,
                        accum_out=gsum_parts[:, f * n_chunks + c : f * n_chunks + c + 1],
                    )

            # ---- combine partials; g = gsum / denom (guarded) ----
            denom = small.tile([_GPC, 1], f32)
            nc.vector.reduce_sum(denom[:], denom_parts[:], axis=AX.X)
            nc.vector.tensor_scalar_max(denom[:], denom[:], 1e-30)
            inv = small.tile([_GPC, 1], f32)
            nc.vector.reciprocal(inv[:], denom[:])
            gsum = small.tile([_GPC, _Z], f32)
            nc.vector.reduce_sum(
                gsum[:],
                gsum_parts[:].rearrange("p (f c) -> p f c", c=n_chunks),
                axis=AX.X,
            )
            g_sb = small.tile([_GPC, _Z], f32)
            nc.vector.tensor_scalar_mul(g_sb[:], gsum[:], inv[:, 0:1])
            nc.sync.dma_start(gout[:], g_sb[:])

            # ---- FiLM MLP: h = relu(g@W1 + b1); gb = h@W2 + b2 ----
            gT_ps = psum.tile([_Z, 128], f32)
            nc.tensor.transpose(gT_ps[:], g_sb[:], ident_sb[:])
            lhsT1 = small.tile([_Z + 1, 128], f32)
            nc.scalar.copy(lhsT1[0:_Z, :], gT_ps[:])
            nc.vector.memset(lhsT1[_Z : _Z + 1, :], 1.0)
            h_ps = psum.tile([_GPC, _H], f32)
            nc.tensor.matmul(h_ps[:], lhsT1[:], w1b_sb[:])
            h_sb = small.tile([_GPC, _H], f32)
            nc.scalar.activation(h_sb[:], h_ps[:], AF.Relu)

            hT_ps = psum.tile([_H, 128], f32)
            nc.tensor.transpose(hT_ps[:], h_sb[:], ident_sb[:])
            lhsT2 = small.tile([_H + 1, 128], f32)
            nc.scalar.copy(lhsT2[0:_H, :], hT_ps[:])
            nc.vector.memset(lhsT2[_H : _H + 1, :], 1.0)
            gb_ps = psum.tile([_GPC, 2 * _Z], f32)
            nc.tensor.matmul(gb_ps[:], lhsT2[:], w2b_sb[:])
            gp1 = small.tile([_GPC, _Z], f32)
            nc.scalar.activation(gp1[:], gb_ps[:, 0:_Z], AF.Identity, bias=1.0)
            beta = small.tile([_GPC, _Z], f32)
            nc.scalar.copy(beta[:], gb_ps[:, _Z : 2 * _Z])

            # ---- pass 2: z_mod = z * (1+gamma) + beta, per-feature ACT ----
            for c in range(n_chunks):
                zt = zpool.tile([_GPC, Lc, _Z], f32, tag="z")
                nc.sync.dma_start(zt[:], zin[:, c * Lc : (c + 1) * Lc, :])
                for f in range(_Z):
                    nc.scalar.activation(
                        zt[:, :, f], zt[:, :, f], AF.Identity,
                        bias=beta[:, f : f + 1], scale=gp1[:, f : f + 1],
                    )
                nc.sync.dma_start(zmod[:, c * Lc : (c + 1) * Lc, :], zt[:])

    nc.compile()
    return nc


def kernel(z_local, Wg, bg, W1, b1, W2, b2, batch_vec, num_graphs):
    global _last_results
    from concourse.bass_utils import run_bass_kernel_spmd

    z = np.ascontiguousarray(np.asarray(z_local, dtype=np.float32))
    wg = np.asarray(Wg, dtype=np.float32).reshape(-1)
    bgv = float(np.asarray(bg, dtype=np.float32).reshape(-1)[0])
    W1 = np.asarray(W1, dtype=np.float32)
    b1 = np.asarray(b1, dtype=np.float32)
    W2 = np.asarray(W2, dtype=np.float32)
    b2 = np.asarray(b2, dtype=np.float32)
    bv = np.asarray(batch_vec).astype(np.int64)
    B = int(num_graphs)
    N, Zdim = z.shape
    assert Zdim == _Z and B == _NCORES * _GPC

    counts = np.bincount(bv, minlength=B)
    n_chunks = 4
    Lc = -(-int(counts.max()) // n_chunks)
    Lc = ((Lc + 15) // 16) * 16
    L = Lc * n_chunks

    # pack nodes: graph b occupies rows [b*L, b*L + counts[b]); the rest of
    # each row block is poisoned so its gate is -1e30 -> e = 0.
    offsets = np.zeros(B + 1, np.int64)
    np.cumsum(counts, out=offsets[1:])
    dest = bv * L + (np.arange(N, dtype=np.int64) - offsets[bv])
    zpad = np.zeros((B * L, _Z), np.float32)
    valid = np.zeros(B * L, bool)
    valid[dest] = True
    pf = int(np.argmax(np.abs(wg)))
    zpad[~valid, pf] = np.float32(-1e30) / wg[pf]
    zpad[dest] = z
    zpad = zpad.reshape(_NCORES, _GPC, L, _Z)

    w1b = np.ascontiguousarray(np.concatenate([W1, b1[None, :]], 0))
    w2b = np.ascontiguousarray(np.concatenate([W2, b2[None, :]], 0))
    identity = np.eye(128, dtype=np.float32)

    key = (L, Lc, tuple(np.round(wg, 7).tolist()), round(bgv, 7))
    if key not in _prog_cache:
        _prog_cache.clear()
        _prog_cache[key] = _build_program(L, Lc, wg, bgv)
    nc = _prog_cache[key]

    in_maps = [
        {"zin": zpad[c], "w1b": w1b, "w2b": w2b, "ident": identity}
        for c in range(_NCORES)
    ]
    global _last_in_maps
    _last_in_maps = in_maps
    trace = bool(int(os.environ.get("BASSK_TRACE", "0")))
    _last_results = run_bass_kernel_spmd(
        nc, in_maps, list(range(_NCORES)), trace=trace
    )
    res = _last_results.results

    zmod_pad = np.stack([res[c]["zmod"] for c in range(_NCORES)])
    z_mod = zmod_pad.reshape(B * L, _Z)[dest]
    g = np.concatenate([res[c]["gout"] for c in range(_NCORES)], 0)
    return z_mod.astype(np.float32, copy=False), g.astype(np.float32, copy=False)


# revision 8
# speedup vs baseline: 127.6162x; 127.6162x over previous
"""GAE (segment-softmax pooling) + FiLM kernel for 8 trn2 NeuronCores.

Sharding: graphs are padded to a uniform length L and laid out one graph per
SBUF partition row; 128 graphs per core x 8 cores = 1024 graphs. All segment
reductions become free-dim reductions, fully device-local (no collectives).
"""

import os
import numpy as np

_NCORES = 8
_GPC = 128  # graphs per core
_Z = 32
_H = 64

_prog_cache = {}
_last_results = None  # test.py introspects this for profiling info
_last_in_maps = None


def _build_program(L, Lc, wg_vals, bg_val, repeat=1):
    """One SPMD program, run on all 8 cores with different input slabs.

    Per-core tensors:
      zin  [128, L, 32] f32  - padded node features, one graph per partition
      w1b  [33, 64] f32      - [W1; b1]
      w2b  [65, 64] f32      - [W2; b2]
      ident [128, 128] f32   - identity for PE transposes
      zmod [128, L, 32] f32  - output (padded layout)
      gout [128, 32] f32     - per-graph pooled features

    `repeat` re-emits the whole compute body N times (benchmarking only).
    """
    import concourse.bass as bass
    import concourse.mybir as mybir
    from concourse import bacc, tile

    f32 = mybir.dt.float32
    AF = mybir.ActivationFunctionType
    OP = mybir.AluOpType
    AX = mybir.AxisListType
    n_chunks = L // Lc
    assert n_chunks * Lc == L

    nc = bacc.Bacc("TRN2", target_bir_lowering=False, debug=False)

    zin = nc.dram_tensor("zin", [_GPC, L, _Z], f32, kind="ExternalInput")
    w1b = nc.dram_tensor("w1b", [_Z + 1, _H], f32, kind="ExternalInput")
    w2b = nc.dram_tensor("w2b", [_H + 1, 2 * _Z], f32, kind="ExternalInput")
    ident = nc.dram_tensor("ident", [128, 128], f32, kind="ExternalInput")
    zmod = nc.dram_tensor("zmod", [_GPC, L, _Z], f32, kind="ExternalOutput")
    gout = nc.dram_tensor("gout", [_GPC, _Z], f32, kind="ExternalOutput")

    with tile.TileContext(nc) as tc:
        with (
            tc.tile_pool(name="zbuf", bufs=2) as zpool,
            tc.tile_pool(name="ebuf", bufs=2) as epool,
            tc.tile_pool(name="small", bufs=1) as small,
            tc.tile_pool(name="psum", bufs=1, space=bass.MemorySpace.PSUM) as psum,
        ):
            w1b_sb = small.tile([_Z + 1, _H], f32)
            w2b_sb = small.tile([_H + 1, 2 * _Z], f32)
            ident_sb = small.tile([128, 128], f32)
            nc.sync.dma_start(w1b_sb[:], w1b[:])
            nc.sync.dma_start(w2b_sb[:], w2b[:])
            nc.sync.dma_start(ident_sb[:], ident[:])
            bg_col = small.tile([_GPC, 1], f32)
            nc.vector.memset(bg_col[:], float(bg_val))

            for _rep in range(repeat):
                # ---- pass 1: gate -> e -> per-graph partial sums ----
                denom_parts = small.tile([_GPC, n_chunks], f32, tag="dparts")
                gsum_parts = small.tile([_GPC, _Z * n_chunks], f32, tag="gparts")
                for c in range(n_chunks):
                    zt = zpool.tile([_GPC, Lc, _Z], f32, tag="z")
                    nc.sync.dma_start(zt[:], zin[:, c * Lc : (c + 1) * Lc, :])
                    # gate accumulation: acc = sum_f z[:, :, f] * wg[f]
                    acc = epool.tile([_GPC, Lc], f32, tag="acc")
                    nc.vector.tensor_scalar_mul(
                        acc[:], zt[:, :, 0], float(wg_vals[0])
                    )
                    for f in range(1, _Z):
                        nc.vector.scalar_tensor_tensor(
                            acc[:], zt[:, :, f], float(wg_vals[f]), acc[:],
                            OP.mult, OP.add,
                        )
                    # e = exp(gate + bg); denom partial via accum_out
                    e = epool.tile([_GPC, Lc], f32, tag="e")
                    nc.scalar.activation(
                        e[:], acc[:], AF.Exp, bias=bg_col[:, 0:1],
                        accum_out=denom_parts[:, c : c + 1],
                    )
                    # gsum partials: sum_t e * z_f, one fused op per feature
                    # (tensor_tensor_reduce crashes the exec unit on this
                    #  runtime; scalar_tensor_tensor+accum_out is equivalent)
                    scratch = epool.tile([_GPC, Lc], f32, tag="scr")
                    for f in range(_Z):
                        nc.vector.scalar_tensor_tensor(
                            scratch[:], zt[:, :, f], 1.0, e[:],
                            OP.mult, OP.mult,
                            accum_out=gsum_parts[
                                :, f * n_chunks + c : f * n_chunks + c + 1
                            ],
                        )

                # ---- combine partials; g = gsum / denom (guarded) ----
                denom = small.tile([_GPC, 1], f32, tag="denom")
                nc.vector.reduce_sum(denom[:], denom_parts[:], axis=AX.X)
                nc.vector.tensor_scalar_max(denom[:], denom[:], 1e-30)
                inv = small.tile([_GPC, 1], f32, tag="inv")
                nc.vector.reciprocal(inv[:], denom[:])
                gsum = small.tile([_GPC, _Z], f32, tag="gsum")
                nc.vector.reduce_sum(
                    gsum[:],
                    gsum_parts[:].rearrange("p (f c) -> p f c", c=n_chunks),
                    axis=AX.X,
                )
                g_sb = small.tile([_GPC, _Z], f32, tag="gsb")
                nc.vector.tensor_scalar_mul(g_sb[:], gsum[:], inv[:, 0:1])
                nc.sync.dma_start(gout[:], g_sb[:])

                # ---- FiLM MLP ----
                gT_ps = psum.tile([_Z, 128], f32, tag="gT")
                nc.tensor.transpose(gT_ps[:], g_sb[:], ident_sb[:])
                lhsT1 = small.tile([_Z + 1, 128], f32, tag="l1")
                nc.scalar.copy(lhsT1[0:_Z, :], gT_ps[:])
                nc.vector.memset(lhsT1[_Z : _Z + 1, :], 1.0)
                h_ps = psum.tile([_GPC, _H], f32, tag="h")
                nc.tensor.matmul(h_ps[:], lhsT1[:], w1b_sb[:])
                h_sb = small.tile([_GPC, _H], f32, tag="hsb")
                nc.scalar.activation(h_sb[:], h_ps[:], AF.Relu)

                hT_ps = psum.tile([_H, 128], f32, tag="hT")
                nc.tensor.transpose(hT_ps[:], h_sb[:], ident_sb[:])
                lhsT2 = small.tile([_H + 1, 128], f32, tag="l2")
                nc.scalar.copy(lhsT2[0:_H, :], hT_ps[:])
                nc.vector.memset(lhsT2[_H : _H + 1, :], 1.0)
                gb_ps = psum.tile([_GPC, 2 * _Z], f32, tag="gb")
                nc.tensor.matmul(gb_ps[:], lhsT2[:], w2b_sb[:])
                gp1 = small.tile([_GPC, _Z], f32, tag="gp1")
                nc.scalar.activation(gp1[:], gb_ps[:, 0:_Z], AF.Identity, bias=1.0)
                beta = small.tile([_GPC, _Z], f32, tag="beta")
                nc.scalar.copy(beta[:], gb_ps[:, _Z : 2 * _Z])

                # ---- pass 2: z_mod = z*(1+gamma) + beta, per-feature ACT ----
                for c in range(n_chunks):
                    zt = zpool.tile([_GPC, Lc, _Z], f32, tag="z")
                    nc.sync.dma_start(zt[:], zin[:, c * Lc : (c + 1) * Lc, :])
                    for f in range(_Z):
                        nc.scalar.activation(
                            zt[:, :, f], zt[:, :, f], AF.Identity,
                            bias=beta[:, f : f + 1], scale=gp1[:, f : f + 1],
                        )
                    nc.sync.dma_start(zmod[:, c * Lc : (c + 1) * Lc, :], zt[:])

    nc.compile()
    return nc


def kernel(z_local, Wg, bg, W1, b1, W2, b2, batch_vec, num_graphs):
    global _last_results, _last_in_maps
    from concourse.bass_utils import run_bass_kernel_spmd

    z = np.ascontiguousarray(np.asarray(z_local, dtype=np.float32))
    wg = np.asarray(Wg, dtype=np.float32).reshape(-1)
    bgv = float(np.asarray(bg, dtype=np.float32).reshape(-1)[0])
    W1 = np.asarray(W1, dtype=np.float32)
    b1 = np.asarray(b1, dtype=np.float32)
    W2 = np.asarray(W2, dtype=np.float32)
    b2 = np.asarray(b2, dtype=np.float32)
    bv = np.asarray(batch_vec).astype(np.int64)
    B = int(num_graphs)
    N, Zdim = z.shape
    assert Zdim == _Z and B == _NCORES * _GPC

    counts = np.bincount(bv, minlength=B)
    n_chunks = 4
    Lc = -(-int(counts.max()) // n_chunks)
    Lc = ((Lc + 15) // 16) * 16
    L = Lc * n_chunks

    # pack nodes: graph b occupies rows [b*L, b*L + counts[b]); the rest of
    # each row block is poisoned so its gate is -1e30 -> e = 0.
    offsets = np.zeros(B + 1, np.int64)
    np.cumsum(counts, out=offsets[1:])
    dest = bv * L + (np.arange(N, dtype=np.int64) - offsets[bv])
    zpad = np.zeros((B * L, _Z), np.float32)
    valid = np.zeros(B * L, bool)
    valid[dest] = True
    pf = int(np.argmax(np.abs(wg)))
    zpad[~valid, pf] = np.float32(-1e30) / wg[pf]
    zpad[dest] = z
    zpad = zpad.reshape(_NCORES, _GPC, L, _Z)

    w1b = np.ascontiguousarray(np.concatenate([W1, b1[None, :]], 0))
    w2b = np.ascontiguousarray(np.concatenate([W2, b2[None, :]], 0))
    identity = np.eye(128, dtype=np.float32)

    key = (L, Lc, tuple(np.round(wg, 7).tolist()), round(bgv, 7))
    if key not in _prog_cache:
        _prog_cache.clear()
        _prog_cache[key] = _build_program(L, Lc, wg, bgv)
    nc = _prog_cache[key]

    in_maps = [
        {"zin": zpad[c], "w1b": w1b, "w2b": w2b, "ident": identity}
        for c in range(_NCORES)
    ]
    _last_in_maps = in_maps
    trace = bool(int(os.environ.get("BASSK_TRACE", "0")))
    _last_results = run_bass_kernel_spmd(
        nc, in_maps, list(range(_NCORES)), trace=trace
    )
    res = _last_results.results

    zmod_pad = np.stack([res[c]["zmod"] for c in range(_NCORES)])
    z_mod = zmod_pad.reshape(B * L, _Z)[dest]
    g = np.concatenate([res[c]["gout"] for c in range(_NCORES)], 0)
    return z_mod.astype(np.float32, copy=False), g.astype(np.float32, copy=False)
